# revision 3
# baseline (speedup 1.0000x reference)
"""Minibatch discrimination kernel for Trainium2, 8 NeuronCores.

Reference computation:
    mat = einsum('ni,ijk->njk', x, T)            # [N, B, C]
    rd[n,n',b] = sum_c |mat[n,b,c] - mat[n',b,c]|
    o[n,b] = sum_n' exp(-rd[n,n',b])             # includes self term exp(0)=1
    out = concat(x, o)                           # [N, IN+B]

Strategy:
  * The N x N pairwise matrix is symmetric: |mat[n+d] - mat[n]| covers the
    pair (n, n+d) for BOTH output rows n and n+d. We compute each circular
    offset d in 1..128 exactly once: o[n] = self + sum_d E(n,d) + E(n-d,d).
  * Offsets are sharded across the 8 cores: core k handles global offsets
    16k+1 .. 16k+16.  All 8 cores run an IDENTICAL program: the offset base
    16k is realized by feeding each core a second, host-rotated copy of x
    (roll by -16k rows), so the on-device shifted operand mat_rot[n+dl]
    equals mat[n + dl + 16k].
  * GEMM (PE): per c-slice matmuls out_c[b, n] = sum_i T[i,b,c] * x[n,i],
    for both the plain and the rotated x (one rhs of width 512).
  * Pairwise (DVE + ACT): bf16 subtract (2x mode), Abs on scalar engine,
    pairwise-add tree over C on DVE, exp(-rd) on scalar engine, fp32
    accumulation.
  * d=128 would be double-counted by the o2 accumulator (pairs {n, n+128}
    appear once per row already via o1), so core 7's last offset masks its
    o2 contribution with a per-core weight input w2 (1.0 elsewhere).
  * The self term exp(0)=1 is realized by initializing each core's o1
    accumulator to 0.125 (8 cores x 0.125 = 1.0 exactly).
  * The first IN output columns are x itself; each core DMA-passes its
    32-row slice through the device untouched.
"""

import numpy as np
import ml_dtypes
from contextlib import ExitStack

import concourse.bass as bass
import concourse.mybir as mybir
import concourse.tile as tile
from concourse import bacc
from concourse.bass_utils import run_bass_kernel_spmd

N, IN, B, C = 256, 1024, 128, 16
NCORES = 8
DPC = 16          # offsets (deltas) per core
KB = IN // 128    # contraction blocks
ROWS = N // NCORES  # passthrough rows per core

BF16 = mybir.dt.bfloat16
F32 = mybir.dt.float32
AF = mybir.ActivationFunctionType

_cached_nc = None


def _build_program():
    nc = bacc.Bacc("TRN2", target_bir_lowering=False, debug=False)

    Tt = nc.dram_tensor("Tt", [C, IN, B], BF16, kind="ExternalInput").ap()
    xTd = nc.dram_tensor("xTd", [IN, 2 * N], BF16, kind="ExternalInput").ap()
    xk = nc.dram_tensor("xk", [ROWS, IN], F32, kind="ExternalInput").ap()
    w2 = nc.dram_tensor("w2", [B, 1], F32, kind="ExternalInput").ap()
    o_out = nc.dram_tensor("o_out", [B, 2 * N], F32, kind="ExternalOutput").ap()
    y_out = nc.dram_tensor("y_out", [ROWS, IN], F32, kind="ExternalOutput").ap()

    with tile.TileContext(nc) as tc:
        with ExitStack() as ctx:
            const = ctx.enter_context(tc.tile_pool(name="const", bufs=1))
            lhsp = ctx.enter_context(tc.tile_pool(name="lhs", bufs=3))
            psum = ctx.enter_context(tc.tile_pool(name="psum", bufs=4, space="PSUM"))
            work = ctx.enter_context(tc.tile_pool(name="work", bufs=3))

            # ---- passthrough of this core's x rows (independent of the rest)
            xk_t = const.tile([128, ROWS * IN // 128], F32)
            nc.sync.dma_start(xk_t[:], xk.rearrange("r (a f) -> (r a) f", a=4))
            nc.sync.dma_start(y_out.rearrange("r (a f) -> (r a) f", a=4), xk_t[:])

            # ---- load GEMM inputs
            xT_sb = const.tile([128, KB, 2 * N], BF16)
            nc.sync.dma_start(xT_sb[:], xTd.rearrange("(kb p) n -> p kb n", p=128))
            w_sb = const.tile([128, 1], F32)
            nc.sync.dma_start(w_sb[:], w2)

            # mat layout: [128 (b), n, c], bf16.
            matA = const.tile([128, N, C], BF16)            # plain mat
            matB = const.tile([128, N + DPC, C], BF16)      # rotated mat + wrap

            # ---- GEMM: per c-slice, out_c[b, n] = sum_i T[i,b,c] x[n,i]
            for c in range(C):
                lhsT_c = lhsp.tile([128, KB, B], BF16, tag="lhs")
                nc.sync.dma_start(
                    lhsT_c[:], Tt[c].rearrange("(kb p) b -> p kb b", p=128)
                )
                ps = psum.tile([128, 2 * N], F32, tag="ps")
                for kb in range(KB):
                    nc.tensor.matmul(
                        ps[:],
                        lhsT=lhsT_c[:, kb, :],
                        rhs=xT_sb[:, kb, :],
                        start=(kb == 0),
                        stop=(kb == KB - 1),
                    )
                # evacuate (fp32 psum -> bf16 sbuf, strided column writes)
                nc.scalar.copy(matA[:, :, c], ps[:, 0:N])
                nc.vector.tensor_copy(matB[:, 0:N, c], ps[:, N:2 * N])
                nc.vector.tensor_copy(matB[:, N:N + DPC, c], ps[:, N:N + DPC])

            # ---- pairwise offsets
            o1 = const.tile([128, N], F32)
            nc.vector.memset(o1[:], 0.125)
            o2e = const.tile([128, N + DPC], F32)
            nc.vector.memset(o2e[:], 0.0)

            H = C // 2
            for dl in range(1, DPC + 1):
                # two c-halves so the first half's work can overlap the
                # GEMM of the second half (deps are range-precise)
                rh = []
                for h in range(2):
                    cs = h * H
                    d = work.tile([128, N, H], BF16, tag=f"d{h}")
                    nc.vector.tensor_sub(
                        d[:], matB[:, dl:dl + N, cs:cs + H], matA[:, :, cs:cs + H]
                    )
                    dabs = work.tile([128, N, H], BF16, tag=f"dabs{h}")
                    nc.scalar.activation(dabs[:], d[:], AF.Abs)
                    r1 = work.tile([128, N, H // 2], BF16, tag=f"r1{h}")
                    nc.vector.tensor_add(r1[:], dabs[:, :, 0:4], dabs[:, :, 4:8])
                    r2 = work.tile([128, N, H // 4], BF16, tag=f"r2{h}")
                    nc.vector.tensor_add(r2[:], r1[:, :, 0:2], r1[:, :, 2:4])
                    r3 = work.tile([128, N, 1], BF16, tag=f"r3{h}")
                    nc.vector.tensor_add(r3[:], r2[:, :, 0:1], r2[:, :, 1:2])
                    rh.append(r3)
                rd = work.tile([128, N, 1], BF16, tag="rd")
                nc.vector.tensor_add(rd[:], rh[0][:], rh[1][:])
                E = work.tile([128, N], F32, tag="E")
                nc.scalar.activation(E[:], rd[:, :, 0], AF.Exp, scale=-1.0)
                nc.gpsimd.tensor_tensor(o1[:], o1[:], E[:], mybir.AluOpType.add)
                if dl == DPC:
                    Ew = work.tile([128, N], F32, tag="Ew")
                    nc.vector.tensor_scalar_mul(Ew[:], E[:], w_sb[:, 0:1])
                    nc.gpsimd.tensor_tensor(
                        o2e[:, dl:dl + N], o2e[:, dl:dl + N], Ew[:],
                        mybir.AluOpType.add,
                    )
                else:
                    nc.gpsimd.tensor_tensor(
                        o2e[:, dl:dl + N], o2e[:, dl:dl + N], E[:],
                        mybir.AluOpType.add,
                    )

            # ---- fold o2 wraparound and write out
            o2f = const.tile([128, N], F32)
            nc.vector.tensor_copy(o2f[:], o2e[:, 0:N])
            nc.vector.tensor_add(o2f[:, 0:DPC], o2f[:, 0:DPC], o2e[:, N:N + DPC])
            nc.sync.dma_start(o_out[:, 0:N], o1[:])
            nc.sync.dma_start(o_out[:, N:2 * N], o2f[:])

    nc.compile()
    return nc


def _get_program():
    global _cached_nc
    if _cached_nc is None:
        _cached_nc = _build_program()
    return _cached_nc


def make_in_maps(x, T):
    bf16 = ml_dtypes.bfloat16
    Tt = np.ascontiguousarray(T.transpose(2, 0, 1)).astype(bf16)
    xT = x.T
    in_maps = []
    for k in range(NCORES):
        xrotT = np.roll(x, -DPC * k, axis=0).T
        xTd = np.ascontiguousarray(np.concatenate([xT, xrotT], axis=1)).astype(bf16)
        w = np.full((B, 1), 0.0 if k == NCORES - 1 else 1.0, dtype=np.float32)
        xk = np.ascontiguousarray(x[ROWS * k:ROWS * (k + 1)], dtype=np.float32)
        in_maps.append({"Tt": Tt, "xTd": xTd, "w2": w, "xk": xk})
    return in_maps


def assemble(results, out_dtype=np.float32):
    O = np.zeros((B, N), dtype=np.float32)
    ys = []
    for k in range(NCORES):
        out = results[k]["o_out"]
        O += out[:, :N]
        O += np.roll(out[:, N:], DPC * k, axis=1)
        ys.append(results[k]["y_out"])
    o = O.T  # [N, B]
    xfull = np.concatenate(ys, axis=0)  # [N, IN]
    return np.concatenate([xfull, o], axis=1).astype(out_dtype)


def run_cores(x, T, trace=False, **kwargs):
    nc = _get_program()
    in_maps = make_in_maps(np.asarray(x, np.float32), np.asarray(T, np.float32))
    return run_bass_kernel_spmd(
        nc, in_maps, core_ids=list(range(NCORES)), trace=trace, **kwargs
    )


def kernel(x, T):
    res = run_cores(x, T)
    return assemble(res.results)
